# revision 1
# baseline (speedup 1.0000x reference)
"""Single-head attention kernel v3 for Trainium2 (8 NeuronCores, SPMD).

Problem: x[4,4096,1024] f32, padding_mask[4,1,4096] i32, Wk/Wq/Wv[64,1024] f32.
  k/q/v = x @ W.T ; wei = softmax(mask(q k^T / 8)) ; out = wei @ v  -> [4,4096,64]

Sharding: core c = (b = c//2, half = c%2). The host rotates x[b] (and the key
mask) so this core's 2048 queries are always rows 0:2048 -- attention is
permutation-invariant over keys, so rotating keys + key-mask together is
exact.  Each core computes k/v for all 4096 keys and q for its local half,
returning out[2048, 64].  No cross-core exchange (a pairwise AllGather was
tried and cost a ~79us PE-idle barrier; local recompute is cheaper).

Per-core pipeline (all matmuls bf16 -- f32r is power-throttled ~4x on trn2):
  1. x arrives host-converted to bf16; DMA-XBAR transposes (serialized on one
     queue: concurrent XBAR corrupts) load xT [c,t] straight into SBUF.
     Zero PE transposes in the whole kernel.
  2. Projections, two groups of four 512-token blocks: stationary [wq|wk]
     [128c,128] -> PSUM rows 0:64 = q, 64:128 = k, accumulated over 8 cc
     chunks; v via stationary wv [128c,64] -> vT [64,512] (2 blocks/bank).
  3. Copies: q -> qT3 bf16 [64,qb,512]; k -> kstage then an SBUF->SBUF DMA
     hop down to partitions 0:64 (kT3 [64,kc,128]; engines can't cross
     partitions, DMA can); vT -> XBAR transpose -> v_sb [keys,65] with a
     ones column for softmax denominators.
  4. Phase 2, kc-major over 32 key chunks, 4 query blocks inner: scores
     sT[128k,512q] = kT3^T qT3, exp on ACT (scale 1/8, per-key bias
     -1e5*(1-mask) underflows masked keys to exactly 0) -> bf16,
     PV oT[65,512] += v_ext^T exp accumulated over all kc.
  5. Epilogue per qb: oT -> bf16 [80,512] -> XBAR transpose -> [q,65]
     (block-major fold: query = a*128+p), multiply by 1/denominator and the
     query mask -> f32 out.
"""

import sys

if "/opt/trn_rl_repo" not in sys.path:
    sys.path.insert(0, "/opt/trn_rl_repo")

import numpy as np
import ml_dtypes

import concourse.bass as bass
import concourse.mybir as mybir
import concourse.tile as tile
from concourse import bacc
from concourse.bass_utils import run_bass_kernel_spmd

F32 = mybir.dt.float32
BF16 = mybir.dt.bfloat16
FP8 = mybir.dt.float8e4
DR = mybir.MatmulPerfMode.DoubleRow

T = 4096
TL = 2048
C = 1024
H = 64
NCC = 8
NKC = 32
NKCL = 16
NQB = 4
NTT = TL // 128   # 16
NEG = -1.0e5

KBYTES = NKCL * 128 * 2          # 4096 bytes/partition of own kT slab (bf16)
VBYTES = NKCL * 66 * 2           # 2112 bytes/partition of own v slab

USE_DR = False
USE_NOLOAD = False
DEBUG = False


def mm_noload(nc, out, lhsT, rhs, start=True, stop=True, perf_mode=None,
              tile_position=None):
    te = nc.tensor
    keep = {0, 1} if perf_mode == DR else {0}
    ifmap_ap = te.lower_ap(rhs.opt(keep), opt=False)
    weights_ap = te.lower_ap(lhsT.opt(keep), opt=False, for_matmul_weights=True)
    out_ap = te.lower_ap(out)
    if tile_position is None:
        tile_position = (rhs.base_partition(), out.base_partition())
    return te.add_instruction(
        mybir.InstMatmult(
            name=f"I-{nc.next_id()}",
            replication_resolution=0,
            replication_shift_amnt=0,
            replication_num_rows=0,
            start_tensor_calc=start,
            stop_tensor_calc=stop,
            ins=[ifmap_ap, weights_ap],
            outs=[out_ap],
            perf_mode=perf_mode,
            is_transpose=None,
            tile_position=tile_position,
            tile_size=(128, 128),
            ldweights=False,
        )
    )


def _phase1_copies(nc, g, kq_ps, vt_ps, qT3, kT3, kstage, vT_sb, v_stg, v_sb):
    """PSUM -> phase-2 layouts for token group g (tbs 4g..4g+4)."""
    def _copy(eng, out, in_):
        if eng is nc.scalar:
            eng.copy(out, in_)
        else:
            eng.tensor_copy(out, in_)

    engines = [nc.vector, nc.scalar]
    for tl in range(4):
        tb = 4 * g + tl
        if g == 0:  # local queries are always the first 2048 rotated rows
            _copy(engines[tl % 2], qT3[:, tb, :], kq_ps[tl][0:64, :])
        _copy(engines[(tl + 1) % 2], kstage[64:128, tl % 2, :],
              kq_ps[tl][64:128, :])
        nc.gpsimd.dma_start(
            out=kT3[:, 4 * tb:4 * tb + 4, :],
            in_=kstage[64:128, tl % 2, :].rearrange("p (kc f) -> p kc f", kc=4),
        )
        r0 = (tl % 2) * 64
        _copy(engines[tl % 2],
              vT_sb[r0:r0 + 64, (tl // 2) * 512:(tl // 2 + 1) * 512],
              vt_ps[tl // 2][r0:r0 + 64, :])
    for tl in range(4):
        tb = 4 * g + tl
        r0 = (tl % 2) * 64
        c0 = (tl // 2) * 512
        for kcl in range(4):
            nc.sync.dma_start_transpose(
                v_stg[:, 4 * tl + kcl, :],
                vT_sb[r0:r0 + 64, c0 + kcl * 128:c0 + (kcl + 1) * 128],
            )
    nc.gpsimd.tensor_copy(v_sb[:, g * NKCL:(g + 1) * NKCL, 0:64], v_stg)


def build_nc():
    nc = bacc.Bacc("TRN2", target_bir_lowering=False, debug=False, num_devices=8)

    x_d = nc.dram_tensor("x", [T, C], BF16, kind="ExternalInput")
    wkq_d = nc.dram_tensor("wkq", [128, NCC, 128], BF16, kind="ExternalInput")
    wv_d = nc.dram_tensor("wv", [128, NCC, H], BF16, kind="ExternalInput")
    nbias_d = nc.dram_tensor("nbias", [128, NKC], F32, kind="ExternalInput")
    maskq_d = nc.dram_tensor("maskq", [128, NTT], F32, kind="ExternalInput")
    out_d = nc.dram_tensor("out", [TL, H], F32, kind="ExternalOutput")
    dbg = {}
    if DEBUG:
        dbg["kT"] = nc.dram_tensor("dbg_kT", [64, NKC * 128], BF16, kind="ExternalOutput")
        dbg["qT"] = nc.dram_tensor("dbg_qT", [64, NQB * 512], BF16, kind="ExternalOutput")
        dbg["v"] = nc.dram_tensor("dbg_v", [128, NKC * 66], BF16, kind="ExternalOutput")
        dbg["oTT"] = nc.dram_tensor("dbg_oTT", [128, NTT * 80], BF16, kind="ExternalOutput")
        dbg["sT"] = nc.dram_tensor("dbg_sT", [128, 512], F32, kind="ExternalOutput")
        dbg["exp"] = nc.dram_tensor("dbg_exp", [128, 512], BF16, kind="ExternalOutput")
        dbg["oT"] = nc.dram_tensor("dbg_oT", [80, 512], BF16, kind="ExternalOutput")

    with tile.TileContext(nc) as tc:
        with (
            tc.tile_pool(name="const", bufs=1) as const,
            tc.tile_pool(name="persist", bufs=1) as persist,
            tc.tile_pool(name="expp", bufs=6) as expp,
            tc.tile_pool(name="osb", bufs=2) as osb,
            tc.tile_pool(name="small", bufs=4) as small,
            tc.tile_pool(name="P1", bufs=4, space=bass.MemorySpace.PSUM) as P1,
            tc.tile_pool(name="P2", bufs=4, space=bass.MemorySpace.PSUM) as P2,
            tc.tile_pool(name="dram", bufs=1, space="DRAM") as dram,
        ):
            _emit(nc, const, persist, expp, osb, small, P1, P2, dram,
                  x_d, wkq_d, wv_d, nbias_d, maskq_d, out_d, dbg)

    nc.compile()
    return nc


def _emit(nc, const, persist, expp, osb, small, P1, P2, dram,
          x_d, wkq_d, wv_d, nbias_d, maskq_d, out_d, dbg=None):
    # ---------------- constants / persistent tiles ----------------
    wkq_sb = const.tile([128, NCC, 128], BF16)
    wv_sb = const.tile([128, NCC, H], BF16)
    nbias_sb = const.tile([128, NKC], F32)
    maskq_sb = const.tile([128, NTT], F32)
    nc.gpsimd.dma_start(out=wkq_sb, in_=wkq_d.ap())
    nc.gpsimd.dma_start(out=wv_sb, in_=wv_d.ap())
    nc.gpsimd.dma_start(out=nbias_sb, in_=nbias_d.ap())
    nc.gpsimd.dma_start(out=maskq_sb, in_=maskq_d.ap())

    xT_sb = persist.tile([128, NCC, T], BF16)
    kT3 = persist.tile([64, NKC, 128], BF16)
    kstage = persist.tile([128, 2, 4 * 128], BF16)  # k hop, double-buffered
    qT3 = persist.tile([64, NQB, 512], BF16)
    v_sb = persist.tile([128, NKC, 66], BF16)       # [key, 65(+pad)]
    vT_sb = persist.tile([128, 2 * 512], BF16)      # vT staging, parity rows
    oTT = persist.tile([128, NTT, 80], BF16)
    out_acc = persist.tile([128, NTT, H], F32)

    ones_sb = const.tile([128, NKC], BF16)
    nc.gpsimd.memset(ones_sb, 1.0)
    nc.gpsimd.tensor_copy(v_sb[:, :, 64], ones_sb)

    # ---------------- 1) x^T via DMA-XBAR transpose ----------------
    # NOTE: all XBAR transposes go through one engine queue (nc.sync):
    # concurrent XBAR use from two queues corrupts output (shared unit).
    for cc in range(NCC):
        nc.sync.dma_start_transpose(
            xT_sb[:, cc, :], x_d.ap()[:, cc * 128:(cc + 1) * 128]
        )

    # ---------------- 2) projections ----------------
    v_stg = persist.tile([128, NKCL, 64], BF16)
    for g in range(2):
        kq_ps = [P1.tile([128, 512], F32, tag="s", name="kq") for i in range(4)]
        vt_ps = [P2.tile([128, 512], F32, tag="o", name="vt") for i in range(2)]
        for cc in range(NCC):
            first, last = cc == 0, cc == NCC - 1
            for tl in range(4):
                tb = 4 * g + tl
                nc.tensor.matmul(
                    kq_ps[tl],
                    wkq_sb[:, cc, :],
                    xT_sb[:, cc, tb * 512:(tb + 1) * 512],
                    start=first, stop=last,
                )
            for tl in range(4):
                tb = 4 * g + tl
                r0 = (tl % 2) * 64
                nc.tensor.matmul(
                    vt_ps[tl // 2][r0:r0 + 64, :],
                    wv_sb[:, cc, :],
                    xT_sb[:, cc, tb * 512:(tb + 1) * 512],
                    start=first, stop=last,
                )
        _phase1_copies(nc, g, kq_ps, vt_ps, qT3, kT3, kstage, vT_sb, v_stg,
                       v_sb)

    if DEBUG:
        nc.gpsimd.dma_start(out=dbg["kT"].ap(), in_=kT3.rearrange("p a f -> p (a f)"))
        nc.gpsimd.dma_start(out=dbg["qT"].ap(), in_=qT3.rearrange("p a f -> p (a f)"))
        nc.gpsimd.dma_start(out=dbg["v"].ap(), in_=v_sb.rearrange("p a f -> p (a f)"))

    # ---------------- 4) (no exchange: all keys computed locally) ---------

    # ---------------- 5) phase 2 ----------------    # ---------------- 5) phase 2 ----------------
    oT_ps = [P2.tile([128, 512], F32, tag="o", name=f"oT{i}") for i in range(NQB)]
    for kc in range(NKC):
        sT = []
        for qb in range(NQB):
            s = P1.tile([128, 512], F32, tag="s", name="sT")
            nc.tensor.matmul(
                s, kT3[:, kc, :], qT3[:, qb, :],
                start=True, stop=True,
            )
            sT.append(s)
        if DEBUG and kc == 0:
            dbg_s = small.tile([128, 512], F32, name="dbgs")
            nc.vector.tensor_copy(dbg_s, sT[0])
            nc.gpsimd.dma_start(out=dbg["sT"].ap(), in_=dbg_s)
        exps = []
        for qb in range(NQB):
            e = expp.tile([128, 512], BF16, name="exp")
            nc.scalar.activation(
                e, sT[qb], mybir.ActivationFunctionType.Exp,
                bias=nbias_sb[:, kc:kc + 1], scale=0.125,
            )
            exps.append(e)
        if DEBUG and kc == 0:
            nc.gpsimd.dma_start(out=dbg["exp"].ap(), in_=exps[0])
        for qb in range(NQB):
            nc.tensor.matmul(
                oT_ps[qb][0:65, :], v_sb[:, kc, 0:65], exps[qb],
                start=(kc == 0), stop=(kc == NKC - 1),
            )

    # ---------------- 6) epilogue ----------------
    # XBAR fold for 512 columns is block-major: oTT[p, 4*qb+a, j] =
    # oTs[j, a*128+p], i.e. query qb*512 + a*128 + p -- the natural layout.
    for qb in range(NQB):
        oTs = osb.tile([80, 512], BF16)
        nc.vector.tensor_copy(oTs[0:65, :], oT_ps[qb][0:65, :])
        if DEBUG and qb == 0:
            nc.gpsimd.dma_start(out=dbg["oT"].ap(), in_=oTs)
        nc.sync.dma_start_transpose(oTT[:, 4 * qb:4 * qb + 4, :], oTs)
    for tt in range(NTT):
        recip = small.tile([128, 1], F32)
        nc.vector.reciprocal(recip, oTT[:, tt, 64:65])
        nc.vector.tensor_scalar(
            out=out_acc[:, tt, :],
            in0=oTT[:, tt, 0:64],
            scalar1=recip,
            scalar2=maskq_sb[:, tt:tt + 1],
            op0=mybir.AluOpType.mult,
            op1=mybir.AluOpType.mult,
        )
    if DEBUG:
        nc.gpsimd.dma_start(out=dbg["oTT"].ap(), in_=oTT.rearrange("p a f -> p (a f)"))
    nc.gpsimd.dma_start(
        out=out_d.ap().rearrange("(n p) h -> p n h", p=128), in_=out_acc
    )


_NC_CACHE = None


def _get_nc():
    global _NC_CACHE
    if _NC_CACHE is None:
        _NC_CACHE = build_nc()
    return _NC_CACHE


def make_in_maps(x, padding_mask, Wk, Wq, Wv):
    x = np.asarray(x)
    padding_mask = np.asarray(padding_mask)

    def wt(w):  # [64,1024] -> [128, 8, 64]: wt[p, cc, h] = w[h, cc*128+p]
        return np.ascontiguousarray(
            np.asarray(w).T.reshape(NCC, 128, H).transpose(1, 0, 2)
        )

    wkt, wqt, wvt = wt(Wk), wt(Wq), wt(Wv)
    # stationary [wq | wk] -> psum rows 0:64 = q, 64:128 = k
    wkq = np.concatenate([wqt, wkt], axis=2).astype(ml_dtypes.bfloat16)
    wv = wvt.astype(ml_dtypes.bfloat16)

    in_maps = []
    for core in range(8):
        b, half = core // 2, core % 2
        # rotate keys so this core's queries are always rows 0:2048
        # (attention is permutation-invariant over keys when the key mask
        # is rotated identically)
        q0 = half * TL
        xb = np.ascontiguousarray(
            np.roll(x[b], -q0, axis=0)
        ).astype(ml_dtypes.bfloat16)
        m = np.roll(padding_mask[b, 0].astype(np.float32), -q0)
        nbias = np.ascontiguousarray(
            (NEG * (1.0 - m)).reshape(NKC, 128).T
        )
        maskq = np.ascontiguousarray(m[0:TL].reshape(NTT, 128).T)
        in_maps.append({
            "x": xb, "wkq": wkq, "wv": wv,
            "nbias": nbias, "maskq": maskq,
        })
    return in_maps


def kernel(x, padding_mask, Wk, Wq, Wv):
    nc = _get_nc()
    in_maps = make_in_maps(x, padding_mask, Wk, Wq, Wv)
    res = run_bass_kernel_spmd(nc, in_maps, core_ids=list(range(8)), trace=False)
    B = np.asarray(x).shape[0]
    out = np.empty((B, T, H), dtype=np.float32)
    for c in range(8):
        b, half = c // 2, c % 2
        out[b, half * TL:(half + 1) * TL, :] = res.results[c]["out"]
    return out



# revision 2
# speedup vs baseline: 1.4967x; 1.4967x over previous
"""Single-head attention kernel v4 for Trainium2 (8 NeuronCores, SPMD).

Problem: x[4,4096,1024] f32, padding_mask[4,1,4096] i32, Wk/Wq/Wv[64,1024] f32.
  k/q/v = x @ W.T ; wei = softmax(mask(q k^T / 8)) ; out = wei @ v  -> [4,4096,64]

Sharding: core c = (b = c//2, half = c%2). The host rotates x[b] (and the key
mask) so this core's 2048 queries are always rows 0:2048 -- attention is
permutation-invariant over keys when the key mask rotates identically.  Each
core computes k/v for all 4096 keys and q for its local half, returning
out[2048, 64].  No cross-core exchange.

v4 changes vs v3 (265us baseline):
  - x is transposed on the HOST: xT [128, 8, 4096] bf16 DMAs straight in.
    Kills the 8 serialized DMA-XBAR transposes + their phase-1 stalls.
  - ldweights dedupe: one LDWEIGHTS per stationary change (explicit
    nc.tensor.ldweights + noload matmuls) instead of one per matmul.
  - Wide ACT: exp over [128, 2, 512] (1024 free) per instruction -- 2 per kc
    instead of 4, amortizing the 352-cycle ACT instruction overhead.
  - Software-pipelined phase 2 (PV lags scores by one kc): per kc the PE does
    S(A) S(B) then PV(kc-1); ACT(A,kc) overlaps S(B,kc)/PV.  Keeps the PE
    dense so HAM stays at K=8/8 (baseline ran ALL of phase 2 at 1.2 GHz).
  - PSUM: PS pool 2x[128,2,512] (scores, rotation gives kc double-buffer),
    PO pool 4x[128,512] (phase-1 vT / phase-2 oT accumulators).
"""

import sys

if "/opt/trn_rl_repo" not in sys.path:
    sys.path.insert(0, "/opt/trn_rl_repo")

import numpy as np
import ml_dtypes

import concourse.bass as bass
import concourse.mybir as mybir
import concourse.tile as tile
from concourse import bacc
from concourse.bass_utils import run_bass_kernel_spmd

F32 = mybir.dt.float32
BF16 = mybir.dt.bfloat16

T = 4096
TL = 2048
C = 1024
H = 64
NCC = 8
NKC = 32
NKCL = 16
NQB = 4
NTT = TL // 128   # 16
NEG = -1.0e5


def mm_noload(nc, out, lhsT, rhs, start=True, stop=True, tile_position=None):
    te = nc.tensor
    keep = {0}
    ifmap_ap = te.lower_ap(rhs.opt(keep), opt=False)
    weights_ap = te.lower_ap(lhsT.opt(keep), opt=False, for_matmul_weights=True)
    out_ap = te.lower_ap(out)
    if tile_position is None:
        tile_position = (rhs.base_partition(), out.base_partition())
    return te.add_instruction(
        mybir.InstMatmult(
            name=f"I-{nc.next_id()}",
            replication_resolution=0,
            replication_shift_amnt=0,
            replication_num_rows=0,
            start_tensor_calc=start,
            stop_tensor_calc=stop,
            ins=[ifmap_ap, weights_ap],
            outs=[out_ap],
            perf_mode=None,
            is_transpose=None,
            tile_position=tile_position,
            tile_size=(128, 128),
            ldweights=False,
        )
    )


def _phase1_copies(nc, g, kq_pair, vt_ps, qT3, kT3, kstage, vT_sb, v_stg, v_sb):
    """PSUM -> phase-2 layouts for token group g (tbs 4g..4g+4)."""
    def _copy(eng, out, in_):
        if eng is nc.scalar:
            eng.copy(out, in_)
        else:
            eng.tensor_copy(out, in_)

    engines = [nc.vector, nc.scalar]
    for tl in range(4):
        tb = 4 * g + tl
        kq_ps = kq_pair[tl // 2][:, tl % 2, :]
        if g == 0:  # local queries are always the first 2048 rotated rows
            _copy(engines[tl % 2], qT3[:, tb, :], kq_ps[0:64, :])
        _copy(engines[(tl + 1) % 2], kstage[64:128, tl % 2, :],
              kq_ps[64:128, :])
        nc.gpsimd.dma_start(
            out=kT3[:, 4 * tb:4 * tb + 4, :],
            in_=kstage[64:128, tl % 2, :].rearrange("p (kc f) -> p kc f", kc=4),
        )
        r0 = (tl % 2) * 64
        _copy(engines[tl % 2],
              vT_sb[r0:r0 + 64, (tl // 2) * 512:(tl // 2 + 1) * 512],
              vt_ps[tl // 2][r0:r0 + 64, :])
    for tl in range(4):
        r0 = (tl % 2) * 64
        c0 = (tl // 2) * 512
        for kcl in range(4):
            nc.sync.dma_start_transpose(
                v_stg[:, 4 * tl + kcl, :],
                vT_sb[r0:r0 + 64, c0 + kcl * 128:c0 + (kcl + 1) * 128],
            )
    nc.gpsimd.tensor_copy(v_sb[:, g * NKCL:(g + 1) * NKCL, 0:64], v_stg)


def build_nc():
    nc = bacc.Bacc("TRN2", target_bir_lowering=False, debug=False, num_devices=8)

    xt_d = nc.dram_tensor("xt", [128, NCC, T], BF16, kind="ExternalInput")
    wkq_d = nc.dram_tensor("wkq", [128, NCC, 128], BF16, kind="ExternalInput")
    wv_d = nc.dram_tensor("wv", [128, NCC, H], BF16, kind="ExternalInput")
    nbias_d = nc.dram_tensor("nbias", [128, NKC], F32, kind="ExternalInput")
    maskq_d = nc.dram_tensor("maskq", [128, NTT], F32, kind="ExternalInput")
    out_d = nc.dram_tensor("out", [TL, H], F32, kind="ExternalOutput")

    with tile.TileContext(nc) as tc:
        with (
            tc.tile_pool(name="const", bufs=1) as const,
            tc.tile_pool(name="persist", bufs=1) as persist,
            tc.tile_pool(name="expp", bufs=4) as expp,
            tc.tile_pool(name="osb", bufs=2) as osb,
            tc.tile_pool(name="small", bufs=4) as small,
            tc.tile_pool(name="PS", bufs=2, space=bass.MemorySpace.PSUM) as PS,
            tc.tile_pool(name="PO", bufs=4, space=bass.MemorySpace.PSUM) as PO,
        ):
            _emit(nc, const, persist, expp, osb, small, PS, PO,
                  xt_d, wkq_d, wv_d, nbias_d, maskq_d, out_d)

    nc.compile()
    return nc


def _emit(nc, const, persist, expp, osb, small, PS, PO,
          xt_d, wkq_d, wv_d, nbias_d, maskq_d, out_d):
    # ---------------- constants / persistent tiles ----------------
    wkq_sb = const.tile([128, NCC, 128], BF16)
    wv_sb = const.tile([128, NCC, H], BF16)
    nbias_sb = const.tile([128, NKC], F32)
    maskq_sb = const.tile([128, NTT], F32)
    nc.gpsimd.dma_start(out=wkq_sb, in_=wkq_d.ap())
    nc.gpsimd.dma_start(out=wv_sb, in_=wv_d.ap())
    nc.gpsimd.dma_start(out=nbias_sb, in_=nbias_d.ap())
    nc.gpsimd.dma_start(out=maskq_sb, in_=maskq_d.ap())

    xT_sb = persist.tile([128, NCC, T], BF16)
    kT3 = persist.tile([64, NKC, 128], BF16)
    kstage = persist.tile([128, 2, 4 * 128], BF16)  # k hop, double-buffered
    qT3 = persist.tile([64, NQB, 512], BF16)
    v_sb = persist.tile([128, NKC, 66], BF16)       # [key, 65(+pad)]
    vT_sb = persist.tile([128, 2 * 512], BF16)      # vT staging, parity rows
    oTT = persist.tile([128, NTT, 80], BF16)
    out_acc = persist.tile([128, NTT, H], F32)

    ones_sb = const.tile([128, NKC], BF16)
    nc.gpsimd.memset(ones_sb, 1.0)
    nc.gpsimd.tensor_copy(v_sb[:, :, 64], ones_sb)

    # ---------------- 1) x^T loads (host pre-transposed) ----------------
    for cc in range(NCC):
        nc.sync.dma_start(out=xT_sb[:, cc, :], in_=xt_d.ap()[:, cc, :])

    # ---------------- 2) projections ----------------
    v_stg = persist.tile([128, NKCL, 64], BF16)
    for g in range(2):
        kq_pair = [PS.tile([128, 2, 512], F32, tag="s", name="kq")
                   for _ in range(2)]
        vt_ps = [PO.tile([128, 512], F32, tag="o", name="vt") for _ in range(2)]
        for cc in range(NCC):
            first, last = cc == 0, cc == NCC - 1
            nc.tensor.ldweights(wkq_sb[:, cc, :])
            for tl in range(4):
                tb = 4 * g + tl
                mm_noload(
                    nc, kq_pair[tl // 2][:, tl % 2, :],
                    wkq_sb[:, cc, :],
                    xT_sb[:, cc, tb * 512:(tb + 1) * 512],
                    start=first, stop=last,
                )
            for tl in range(4):
                tb = 4 * g + tl
                r0 = (tl % 2) * 64
                nc.tensor.matmul(
                    vt_ps[tl // 2][r0:r0 + 64, :],
                    wv_sb[:, cc, :],
                    xT_sb[:, cc, tb * 512:(tb + 1) * 512],
                    start=first, stop=last,
                )
        _phase1_copies(nc, g, kq_pair, vt_ps, qT3, kT3, kstage, vT_sb, v_stg,
                       v_sb)

    # ---------------- 3) phase 2: kc-major, PV lags scores by one kc ------
    oT_ps = [PO.tile([128, 512], F32, tag="o", name=f"oT{i}")
             for i in range(NQB)]
    prev = None  # (kc, [eA, eB])
    for kc in range(NKC):
        ss = []
        es = []
        nc.tensor.ldweights(kT3[:, kc, :])
        for grp in range(2):
            s = PS.tile([128, 2, 512], F32, tag="s", name="sT")
            for j in range(2):
                mm_noload(
                    nc, s[:, j, :], kT3[:, kc, :], qT3[:, 2 * grp + j, :],
                    start=True, stop=True,
                )
            e = expp.tile([128, 2, 512], BF16, name="exp")
            nc.scalar.activation(
                e, s, mybir.ActivationFunctionType.Exp,
                bias=nbias_sb[:, kc:kc + 1], scale=0.125,
            )
            ss.append(s)
            es.append(e)
        if prev is not None:
            pkc, pes = prev
            nc.tensor.ldweights(v_sb[:, pkc, 0:65])
            for qb in range(NQB):
                mm_noload(
                    nc, oT_ps[qb][0:65, :],
                    v_sb[:, pkc, 0:65], pes[qb // 2][:, qb % 2, :],
                    start=(pkc == 0), stop=(pkc == NKC - 1),
                )
        prev = (kc, es)
    # tail PV for kc = NKC-1
    pkc, pes = prev
    nc.tensor.ldweights(v_sb[:, pkc, 0:65])
    for qb in range(NQB):
        mm_noload(
            nc, oT_ps[qb][0:65, :],
            v_sb[:, pkc, 0:65], pes[qb // 2][:, qb % 2, :],
            start=(pkc == 0), stop=(pkc == NKC - 1),
        )

    # ---------------- 4) epilogue ----------------
    # XBAR fold for 512 columns is block-major: oTT[p, 4*qb+a, j] =
    # oTs[j, a*128+p], i.e. query qb*512 + a*128 + p -- the natural layout.
    for qb in range(NQB):
        oTs = osb.tile([80, 512], BF16)
        nc.vector.tensor_copy(oTs[0:65, :], oT_ps[qb][0:65, :])
        nc.sync.dma_start_transpose(oTT[:, 4 * qb:4 * qb + 4, :], oTs)
    for tt in range(NTT):
        recip = small.tile([128, 1], F32)
        nc.vector.reciprocal(recip, oTT[:, tt, 64:65])
        nc.vector.tensor_scalar(
            out=out_acc[:, tt, :],
            in0=oTT[:, tt, 0:64],
            scalar1=recip,
            scalar2=maskq_sb[:, tt:tt + 1],
            op0=mybir.AluOpType.mult,
            op1=mybir.AluOpType.mult,
        )
    nc.gpsimd.dma_start(
        out=out_d.ap().rearrange("(n p) h -> p n h", p=128), in_=out_acc
    )


_NC_CACHE = None


def _get_nc():
    global _NC_CACHE
    if _NC_CACHE is None:
        _NC_CACHE = build_nc()
    return _NC_CACHE


def make_in_maps(x, padding_mask, Wk, Wq, Wv):
    x = np.asarray(x)
    padding_mask = np.asarray(padding_mask)

    def wt(w):  # [64,1024] -> [128, 8, 64]: wt[p, cc, h] = w[h, cc*128+p]
        return np.ascontiguousarray(
            np.asarray(w).T.reshape(NCC, 128, H).transpose(1, 0, 2)
        )

    wkt, wqt, wvt = wt(Wk), wt(Wq), wt(Wv)
    # stationary [wq | wk] -> psum rows 0:64 = q, 64:128 = k
    wkq = np.concatenate([wqt, wkt], axis=2).astype(ml_dtypes.bfloat16)
    wv = wvt.astype(ml_dtypes.bfloat16)

    in_maps = []
    for core in range(8):
        b, half = core // 2, core % 2
        # rotate keys so this core's queries are always rows 0:2048
        # (attention is permutation-invariant over keys when the key mask
        # is rotated identically)
        q0 = half * TL
        xb = np.roll(x[b], -q0, axis=0).astype(ml_dtypes.bfloat16)
        # host transpose: xt[p, cc, t] = xb[t, cc*128+p]
        xt = np.ascontiguousarray(
            xb.T.reshape(NCC, 128, T).transpose(1, 0, 2)
        )
        m = np.roll(padding_mask[b, 0].astype(np.float32), -q0)
        nbias = np.ascontiguousarray(
            (NEG * (1.0 - m)).reshape(NKC, 128).T
        )
        maskq = np.ascontiguousarray(m[0:TL].reshape(NTT, 128).T)
        in_maps.append({
            "xt": xt, "wkq": wkq, "wv": wv,
            "nbias": nbias, "maskq": maskq,
        })
    return in_maps


def kernel(x, padding_mask, Wk, Wq, Wv):
    nc = _get_nc()
    in_maps = make_in_maps(x, padding_mask, Wk, Wq, Wv)
    res = run_bass_kernel_spmd(nc, in_maps, core_ids=list(range(8)), trace=False)
    B = np.asarray(x).shape[0]
    out = np.empty((B, T, H), dtype=np.float32)
    for c in range(8):
        b, half = c // 2, c % 2
        out[b, half * TL:(half + 1) * TL, :] = res.results[c]["out"]
    return out


# revision 4
# speedup vs baseline: 2.0585x; 1.3754x over previous
"""Single-head attention kernel v5 for Trainium2 (8 NeuronCores, SPMD).

Problem: x[4,4096,1024] f32, padding_mask[4,1,4096] i32, Wk/Wq/Wv[64,1024] f32.
  k/q/v = x @ W.T ; wei = softmax(mask(q k^T / 8)) ; out = wei @ v  -> [4,4096,64]

Sharding: core c = (b = c//2, half = c%2). The host rotates x[b] (and the key
mask) so this core's 2048 queries are always rows 0:2048 -- attention is
permutation-invariant over keys when the key mask rotates identically.  Each
core computes k/v for all 4096 keys and q for its local half, returning
out[2048, 64].  No cross-core exchange.

v5 changes vs v4 (177us):
  - Stationary is [wk | wq]: k lands at PSUM partitions 0:64 -> direct engine
    copy into kT3 (no kstage staging + DMA hop).  q (g0 only) takes the small
    hop (qstage -> DMA down to partitions 0:64) instead.
  - v transposes batched: 4 XBAR transposes of [64,512] per group straight
    into v_sb slices (vs 16 of [64,128] + v_stg + gpsimd copy).  v4's 19us
    phase-1 stall was head-of-line blocking behind these on the sync queue.
  - x DMA split into per-(cc, half) chunks, g0 halves first, so g0
    projections start after ~1.5us of DMA.
  - Epilogue: one batched oT transpose; recip+mask fused into one scale
    vector; per-tt multiplies split across vector/scalar engines.
"""

import sys

if "/opt/trn_rl_repo" not in sys.path:
    sys.path.insert(0, "/opt/trn_rl_repo")

import numpy as np
import ml_dtypes

import concourse.bass as bass
import concourse.mybir as mybir
import concourse.tile as tile
from concourse import bacc
from concourse.bass_utils import run_bass_kernel_spmd

F32 = mybir.dt.float32
BF16 = mybir.dt.bfloat16

T = 4096
TL = 2048
C = 1024
H = 64
NCC = 8
NKC = 32
NQB = 4
NTT = TL // 128   # 16
NEG = -1.0e5


def mm_noload(nc, out, lhsT, rhs, start=True, stop=True, tile_position=None):
    te = nc.tensor
    keep = {0}
    ifmap_ap = te.lower_ap(rhs.opt(keep), opt=False)
    weights_ap = te.lower_ap(lhsT.opt(keep), opt=False, for_matmul_weights=True)
    out_ap = te.lower_ap(out)
    if tile_position is None:
        tile_position = (rhs.base_partition(), out.base_partition())
    return te.add_instruction(
        mybir.InstMatmult(
            name=f"I-{nc.next_id()}",
            replication_resolution=0,
            replication_shift_amnt=0,
            replication_num_rows=0,
            start_tensor_calc=start,
            stop_tensor_calc=stop,
            ins=[ifmap_ap, weights_ap],
            outs=[out_ap],
            perf_mode=None,
            is_transpose=None,
            tile_position=tile_position,
            tile_size=(128, 128),
            ldweights=False,
        )
    )


def _phase1_copies(nc, g, kq_pair, vt_ps, qstage, qT3, kT3, vT_sb, v_sb):
    """PSUM -> phase-2 layouts for token group g (tbs 4g..4g+4)."""
    def _copy(eng, out, in_):
        if eng is nc.scalar:
            eng.copy(out, in_)
        else:
            eng.tensor_copy(out, in_)

    engines = [nc.vector, nc.scalar]
    for tl in range(4):
        tb = 4 * g + tl
        kq = kq_pair[tl // 2][:, tl % 2, :]
        # k at partitions 0:64 -> straight into kT3 (cast f32->bf16)
        _copy(engines[tl % 2], kT3[:, 4 * tb:4 * tb + 4, :],
              kq[0:64, :].rearrange("p (kc f) -> p kc f", kc=4))
        if g == 0:  # local queries: hop partitions 64:128 -> 0:64
            _copy(engines[(tl + 1) % 2], qstage[64:128, tl, :], kq[64:128, :])
            nc.gpsimd.dma_start(out=qT3[:, tl, :], in_=qstage[64:128, tl, :])
        r0 = (tl % 2) * 64
        c0 = (tl // 2) * 512
        _copy(engines[(tl + 1) % 2], vT_sb[r0:r0 + 64, c0:c0 + 512],
              vt_ps[tl // 2][r0:r0 + 64, :])
    # batched v transposes straight into v_sb: keys for (r0,c0) block are
    # tb*512:(tb+1)*512 with tb = 4g+tl, i.e. kc chunks 4*tb..4*tb+4
    for tl in range(4):
        r0 = (tl % 2) * 64
        c0 = (tl // 2) * 512
        kc0 = g * 16 + 4 * tl
        nc.sync.dma_start_transpose(
            v_sb[:, kc0:kc0 + 4, 0:64], vT_sb[r0:r0 + 64, c0:c0 + 512]
        )


def build_nc():
    nc = bacc.Bacc("TRN2", target_bir_lowering=False, debug=False, num_devices=8)

    xt_d = nc.dram_tensor("xt", [128, NCC, T], BF16, kind="ExternalInput")
    wkq_d = nc.dram_tensor("wkq", [128, NCC, 128], BF16, kind="ExternalInput")
    wv_d = nc.dram_tensor("wv", [128, NCC, H], BF16, kind="ExternalInput")
    nbias_d = nc.dram_tensor("nbias", [128, NKC], F32, kind="ExternalInput")
    maskq_d = nc.dram_tensor("maskq", [128, NTT], F32, kind="ExternalInput")
    out_d = nc.dram_tensor("out", [TL, H], F32, kind="ExternalOutput")

    with tile.TileContext(nc) as tc:
        with (
            tc.tile_pool(name="const", bufs=1) as const,
            tc.tile_pool(name="persist", bufs=1) as persist,
            tc.tile_pool(name="expp", bufs=4) as expp,
            tc.tile_pool(name="osb", bufs=1) as osb,
            tc.tile_pool(name="small", bufs=4) as small,
            tc.tile_pool(name="PS", bufs=2, space=bass.MemorySpace.PSUM) as PS,
            tc.tile_pool(name="PO", bufs=4, space=bass.MemorySpace.PSUM) as PO,
        ):
            _emit(nc, const, persist, expp, osb, small, PS, PO,
                  xt_d, wkq_d, wv_d, nbias_d, maskq_d, out_d)

    nc.compile()
    return nc


def _emit(nc, const, persist, expp, osb, small, PS, PO,
          xt_d, wkq_d, wv_d, nbias_d, maskq_d, out_d):
    # ---------------- constants / persistent tiles ----------------
    wkq_sb = const.tile([128, NCC, 128], BF16)
    wv_sb = const.tile([128, NCC, H], BF16)
    nbias_sb = const.tile([128, NKC], F32)
    maskq_sb = const.tile([128, NTT], F32)
    nc.gpsimd.dma_start(out=wkq_sb, in_=wkq_d.ap())
    nc.gpsimd.dma_start(out=wv_sb, in_=wv_d.ap())
    nc.gpsimd.dma_start(out=nbias_sb, in_=nbias_d.ap())
    nc.gpsimd.dma_start(out=maskq_sb, in_=maskq_d.ap())

    xT_sb = persist.tile([128, NCC, T], BF16)
    kT3 = persist.tile([64, NKC, 128], BF16)
    qT3 = persist.tile([64, NQB, 512], BF16)
    qstage = persist.tile([128, NQB, 512], BF16)
    v_sb = persist.tile([128, NKC, 66], BF16)       # [key, 65(+pad)]
    vT_sb = persist.tile([128, 2 * 512], BF16)      # vT staging, parity rows
    oTT = persist.tile([128, NTT, 80], BF16)
    out_acc = persist.tile([128, NTT, H], F32)

    ones_sb = const.tile([128, NKC], BF16)
    nc.gpsimd.memset(ones_sb, 1.0)
    nc.gpsimd.tensor_copy(v_sb[:, :, 64], ones_sb)

    # ---------------- 1) x^T loads (host pre-transposed), g0 halves first --
    for g in range(2):
        for cc in range(NCC):
            nc.sync.dma_start(
                out=xT_sb[:, cc, g * TL:(g + 1) * TL],
                in_=xt_d.ap()[:, cc, g * TL:(g + 1) * TL],
            )

    # ---------------- 2) projections ----------------
    for g in range(2):
        kq_pair = [PS.tile([128, 2, 512], F32, tag="s", name="kq")
                   for _ in range(2)]
        vt_ps = [PO.tile([128, 512], F32, tag="o", name="vt") for _ in range(2)]
        for cc in range(NCC):
            first, last = cc == 0, cc == NCC - 1
            nc.tensor.ldweights(wkq_sb[:, cc, :])
            for tl in range(4):
                tb = 4 * g + tl
                mm_noload(
                    nc, kq_pair[tl // 2][:, tl % 2, :],
                    wkq_sb[:, cc, :],
                    xT_sb[:, cc, tb * 512:(tb + 1) * 512],
                    start=first, stop=last,
                )
            for tl in range(4):
                tb = 4 * g + tl
                r0 = (tl % 2) * 64
                nc.tensor.matmul(
                    vt_ps[tl // 2][r0:r0 + 64, :],
                    wv_sb[:, cc, :],
                    xT_sb[:, cc, tb * 512:(tb + 1) * 512],
                    start=first, stop=last,
                )
        _phase1_copies(nc, g, kq_pair, vt_ps, qstage, qT3, kT3, vT_sb, v_sb)

    # ---------------- 3) phase 2: kc-major, PV lags scores by one kc ------
    oT_ps = [PO.tile([128, 512], F32, tag="o", name=f"oT{i}")
             for i in range(NQB)]
    prev = None  # (kc, [eA, eB])
    for kc in range(NKC):
        es = []
        nc.tensor.ldweights(kT3[:, kc, :])
        for grp in range(2):
            s = PS.tile([128, 2, 512], F32, tag="s", name="sT")
            for j in range(2):
                mm_noload(
                    nc, s[:, j, :], kT3[:, kc, :], qT3[:, 2 * grp + j, :],
                    start=True, stop=True,
                )
            e = expp.tile([128, 2, 512], BF16, name="exp")
            nc.scalar.activation(
                e, s, mybir.ActivationFunctionType.Exp,
                bias=nbias_sb[:, kc:kc + 1], scale=0.125,
            )
            es.append(e)
        if prev is not None:
            pkc, pes = prev
            nc.tensor.ldweights(v_sb[:, pkc, 0:65])
            for qb in range(NQB):
                mm_noload(
                    nc, oT_ps[qb][0:65, :],
                    v_sb[:, pkc, 0:65], pes[qb // 2][:, qb % 2, :],
                    start=(pkc == 0), stop=(pkc == NKC - 1),
                )
        prev = (kc, es)
    pkc, pes = prev
    nc.tensor.ldweights(v_sb[:, pkc, 0:65])
    for qb in range(NQB):
        mm_noload(
            nc, oT_ps[qb][0:65, :],
            v_sb[:, pkc, 0:65], pes[qb // 2][:, qb % 2, :],
            start=(pkc == 0), stop=(pkc == NKC - 1),
        )

    # ---------------- 4) epilogue ----------------
    # Batched XBAR fold: oTT[p, 4*qb+b, j] = oTs[j, qb, b*128+p], i.e.
    # query = qb*512 + b*128 + p = tt*128 + p with tt = 4*qb+b.
    oTs = osb.tile([80, NQB, 512], BF16)
    engines = [nc.vector, nc.scalar]
    for qb in range(NQB):
        if qb % 2 == 0:
            nc.vector.tensor_copy(oTs[0:65, qb, :], oT_ps[qb][0:65, :])
        else:
            nc.scalar.copy(oTs[0:65, qb, :], oT_ps[qb][0:65, :])
    nc.sync.dma_start_transpose(
        oTT, oTs.rearrange("p a f -> p (a f)")
    )
    recip_all = small.tile([128, NTT], F32)
    scale_all = small.tile([128, NTT], F32)
    nc.vector.reciprocal(
        recip_all, oTT[:, :, 64:65].rearrange("p a one -> p (a one)")
    )
    nc.vector.tensor_tensor(
        scale_all, recip_all, maskq_sb, mybir.AluOpType.mult
    )
    for tt in range(NTT):
        if tt % 2 == 0:
            nc.vector.tensor_scalar(
                out=out_acc[:, tt, :],
                in0=oTT[:, tt, 0:64],
                scalar1=scale_all[:, tt:tt + 1],
                scalar2=None,
                op0=mybir.AluOpType.mult,
            )
        else:
            nc.scalar.mul(out_acc[:, tt, :], oTT[:, tt, 0:64],
                          scale_all[:, tt:tt + 1])
    nc.gpsimd.dma_start(
        out=out_d.ap().rearrange("(n p) h -> p n h", p=128), in_=out_acc
    )


_NC_CACHE = None


def _get_nc():
    global _NC_CACHE
    if _NC_CACHE is None:
        _NC_CACHE = build_nc()
    return _NC_CACHE


def make_in_maps(x, padding_mask, Wk, Wq, Wv):
    x = np.asarray(x)
    padding_mask = np.asarray(padding_mask)

    def wt(w):  # [64,1024] -> [128, 8, 64]: wt[p, cc, h] = w[h, cc*128+p]
        return np.ascontiguousarray(
            np.asarray(w).T.reshape(NCC, 128, H).transpose(1, 0, 2)
        )

    wkt, wqt, wvt = wt(Wk), wt(Wq), wt(Wv)
    # stationary [wk | wq] -> psum rows 0:64 = k, 64:128 = q
    wkq = np.concatenate([wkt, wqt], axis=2).astype(ml_dtypes.bfloat16)
    wv = wvt.astype(ml_dtypes.bfloat16)

    in_maps = []
    for core in range(8):
        b, half = core // 2, core % 2
        # rotate keys so this core's queries are always rows 0:2048
        # (attention is permutation-invariant over keys when the key mask
        # is rotated identically)
        q0 = half * TL
        xb = np.roll(x[b], -q0, axis=0).astype(ml_dtypes.bfloat16)
        # host transpose: xt[p, cc, t] = xb[t, cc*128+p]
        xt = np.ascontiguousarray(
            xb.T.reshape(NCC, 128, T).transpose(1, 0, 2)
        )
        m = np.roll(padding_mask[b, 0].astype(np.float32), -q0)
        nbias = np.ascontiguousarray(
            (NEG * (1.0 - m)).reshape(NKC, 128).T
        )
        maskq = np.ascontiguousarray(m[0:TL].reshape(NTT, 128).T)
        in_maps.append({
            "xt": xt, "wkq": wkq, "wv": wv,
            "nbias": nbias, "maskq": maskq,
        })
    return in_maps


def kernel(x, padding_mask, Wk, Wq, Wv):
    nc = _get_nc()
    in_maps = make_in_maps(x, padding_mask, Wk, Wq, Wv)
    res = run_bass_kernel_spmd(nc, in_maps, core_ids=list(range(8)), trace=False)
    B = np.asarray(x).shape[0]
    out = np.empty((B, T, H), dtype=np.float32)
    for c in range(8):
        b, half = c // 2, c % 2
        out[b, half * TL:(half + 1) * TL, :] = res.results[c]["out"]
    return out
